# revision 3
# baseline (speedup 1.0000x reference)
"""ChannelAwareAttentionModule TRN2 kernel.

Math (per sample s, all biases are no-ops because InstanceNorm removes them):
  thetaN/phiN/gN = relu(instnorm(w @ x))        [Ci=128, N=4096]
  f = thetaN @ phiN^T                           [128, 128]
  attn = softmax(f, axis=1)
  y = attn @ gN                                 [128, 4096]
  y_view[ci, q*128+r] = y[r, 32*ci+q]           (permute+reshape view)
  out = instnorm(W_w @ y_view) + x              [256, 4096]

Sharding: data-parallel over batch, 2 samples per core, 8 cores.

Layouts on core:
  x_sb      [128, 2, 4096]  (2 chunks of 128 input channels)
  projN     [128, 4096]     per projection, f32r, produced by fused
                            instnorm+relu eviction from PSUM
  thetaT/phiT [128, 32, 128] transposed tiles (PE transpose + evict)
  Z = y_view [128, 4096]    built directly: block q = (gN[:, q::32])^T @ attnT
  final conv + instnorm fused into PSUM eviction, residual add from x_sb
"""
import sys

sys.path.insert(0, "/opt/trn_rl_repo")

import numpy as np

import concourse.bass as bass
import concourse.bacc as bacc
import concourse.tile as tile
from concourse import mybir
from concourse.bass_utils import run_bass_kernel_spmd
from concourse.masks import make_identity

N_CORES = 8
B, C, CI, H, W = 16, 256, 128, 64, 64
N = H * W  # 4096
B_LOC = B // N_CORES  # 2 samples per core
KCH = C // 128  # 2 contraction chunks of the input channels
NT = N // 128  # 32 column tiles
FCH = N // 512  # 8 psum-bank-sized chunks
EPS = 1e-5

F32 = mybir.dt.float32
F32R = mybir.dt.float32r

_CACHE = {}


def build_nc():
    nc = bacc.Bacc("TRN2", target_bir_lowering=False)

    x_ext = nc.declare_dram_parameter("x", [B_LOC, C, N], F32R, isOutput=False)
    # stacked projection weights, host layout [128, KCH, 3, 128] = [c128, k, proj, ci]
    w_ext = nc.declare_dram_parameter("w3", [128, KCH, 3, CI], F32R, isOutput=False)
    ww_ext = nc.declare_dram_parameter("ww", [CI, C], F32R, isOutput=False)
    out_ext = nc.declare_dram_parameter("out", [B_LOC, C, N], F32, isOutput=True)

    with tile.TileContext(nc) as tc:
        from contextlib import ExitStack

        with ExitStack() as ctx:
            consts = ctx.enter_context(tc.tile_pool(name="consts", bufs=1))
            xpool = ctx.enter_context(tc.tile_pool(name="xpool", bufs=2))
            pn = ctx.enter_context(tc.tile_pool(name="pn", bufs=2))
            gn = ctx.enter_context(tc.tile_pool(name="gn", bufs=1))
            pT = ctx.enter_context(tc.tile_pool(name="pT", bufs=2))
            zpool = ctx.enter_context(tc.tile_pool(name="zpool", bufs=1))
            small = ctx.enter_context(tc.tile_pool(name="small", bufs=12))
            banks = ctx.enter_context(tc.tile_pool(name="banks", bufs=8, space="PSUM"))

            # ---- constants ----
            ident32 = consts.tile([128, 128], F32)
            make_identity(nc, ident32[:])
            w_sb = consts.tile([128, KCH, 3, CI], F32R)
            nc.sync.dma_start(w_sb[:], w_ext[:])
            ww_sb = consts.tile([CI, C], F32R)
            nc.sync.dma_start(ww_sb[:], ww_ext[:])
            eps_t = consts.tile([128, 1], F32)
            nc.vector.memset(eps_t[:], EPS)

            def rstd_negmr(mv):
                """mv [128,2] = (mean, var) -> (rstd, -mean*rstd) [128,1] each."""
                rstd = small.tile([128, 1], F32, tag="rstd")
                nc.scalar.activation(
                    rstd[:], mv[:, 1:2], mybir.ActivationFunctionType.Sqrt,
                    bias=eps_t[:], scale=1.0,
                )
                nc.vector.reciprocal(rstd[:], rstd[:])
                negmr = small.tile([128, 1], F32, tag="negmr")
                nc.vector.tensor_mul(negmr[:], mv[:, 0:1], rstd[:])
                nc.scalar.mul(negmr[:], negmr[:], -1.0)
                return rstd, negmr

            for s in range(B_LOC):
                # ---- load x ----
                x_sb = xpool.tile([128, KCH, N], F32R, tag="x")
                for k in range(KCH):
                    nc.sync.dma_start(x_sb[:, k, :], x_ext[s, 128 * k:128 * (k + 1), :])

                # ---- three projections with fused instnorm+relu ----
                projs = []
                for p in range(3):
                    pool = gn if p == 2 else pn
                    ps_list = []
                    stats = small.tile([128, FCH, 6], F32, tag="stats")
                    for fc in range(FCH):
                        ps = banks.tile([128, 512], F32, tag="bank")
                        for k in range(KCH):
                            nc.tensor.matmul(
                                ps[:], w_sb[:, k, p, :],
                                x_sb[:, k, 512 * fc:512 * (fc + 1)],
                                start=(k == 0), stop=(k == KCH - 1),
                            )
                        nc.vector.bn_stats(stats[:, fc, :], ps[:])
                        ps_list.append(ps)
                    mv = small.tile([128, 2], F32, tag="mv")
                    nc.vector.bn_aggr(mv[:], stats[:])
                    rstd, negmr = rstd_negmr(mv)
                    projN = pool.tile(
                        [128, N], F32R if p == 2 else F32,
                        tag=("g" if p == 2 else "pn"))
                    for fc in range(FCH):
                        nc.scalar.activation(
                            projN[:, 512 * fc:512 * (fc + 1)], ps_list[fc][:],
                            mybir.ActivationFunctionType.Relu,
                            bias=negmr[:], scale=rstd[:],
                        )
                    projs.append(projN)
                thetaN, phiN, gN = projs

                # ---- transpose thetaN / phiN into [n, i] tiles ----
                thetaT = pT.tile([128, NT, 128], F32R, tag="pT")
                phiT = pT.tile([128, NT, 128], F32R, tag="pT")
                for src, dst in ((thetaN, thetaT), (phiN, phiT)):
                    for t in range(NT):
                        tp = banks.tile([128, 128], F32, tag="bank")
                        nc.tensor.transpose(
                            tp[:], src[:, 128 * t:128 * (t + 1)], ident32[:]
                        )
                        if t % 3 == 2:
                            nc.scalar.copy(dst[:, t, :], tp[:])
                        else:
                            nc.vector.tensor_copy(dst[:, t, :], tp[:])

                # ---- gram f = thetaN @ phiN^T, softmax rows ----
                f_ps = banks.tile([128, 128], F32, tag="bank")
                for t in range(NT):
                    nc.tensor.matmul(
                        f_ps[:], thetaT[:, t, :], phiT[:, t, :],
                        start=(t == 0), stop=(t == NT - 1),
                    )
                negmax = small.tile([128, 1], F32, tag="negmax")
                nc.vector.tensor_reduce(
                    negmax[:], f_ps[:], axis=mybir.AxisListType.X,
                    op=mybir.AluOpType.max, negate=True,
                )
                attn_e = small.tile([128, 128], F32, tag="attn_e")
                sumexp = small.tile([128, 1], F32, tag="sumexp")
                nc.scalar.activation(
                    attn_e[:], f_ps[:], mybir.ActivationFunctionType.Exp,
                    bias=negmax[:], scale=1.0, accum_out=sumexp[:],
                )
                rsum = small.tile([128, 1], F32, tag="rsum")
                nc.vector.reciprocal(rsum[:], sumexp[:])
                attn_n = small.tile([128, 128], F32, tag="attn_n")
                nc.vector.tensor_scalar_mul(attn_n[:], attn_e[:], rsum[:])
                at_ps = banks.tile([128, 128], F32, tag="bank")
                nc.tensor.transpose(at_ps[:], attn_n[:], ident32[:])
                attnT = small.tile([128, 128], F32R, tag="attnT")
                nc.vector.tensor_copy(attnT[:], at_ps[:])

                # ---- Z = y_view, block q = (gN[:, q::32])^T @ attnT ----
                Z = zpool.tile([128, N], F32R, tag="z")
                g3 = gN[:].rearrange("p (c q) -> p c q", q=NT)
                for q in range(NT):
                    zp = banks.tile([128, 128], F32, tag="bank")
                    nc.tensor.matmul(
                        zp[:], g3[:, :, q], attnT[:], start=True, stop=True
                    )
                    if q % 3 == 2:
                        nc.scalar.copy(Z[:, 128 * q:128 * (q + 1)], zp[:])
                    else:
                        nc.vector.tensor_copy(Z[:, 128 * q:128 * (q + 1)], zp[:])

                # ---- final conv + instnorm (fused evict) + residual ----
                for oc in range(KCH):
                    ps_list = []
                    stats2 = small.tile([128, FCH, 6], F32, tag="stats2")
                    for fc in range(FCH):
                        ps = banks.tile([128, 512], F32, tag="bank")
                        nc.tensor.matmul(
                            ps[:], ww_sb[:, 128 * oc:128 * (oc + 1)],
                            Z[:, 512 * fc:512 * (fc + 1)],
                            start=True, stop=True,
                        )
                        nc.vector.bn_stats(stats2[:, fc, :], ps[:])
                        ps_list.append(ps)
                    mv2 = small.tile([128, 2], F32, tag="mv2")
                    nc.vector.bn_aggr(mv2[:], stats2[:])
                    rstd2, negmr2 = rstd_negmr(mv2)
                    normed = pn.tile([128, N], F32, tag="pn")
                    for fc in range(FCH):
                        nc.scalar.activation(
                            normed[:, 512 * fc:512 * (fc + 1)], ps_list[fc][:],
                            mybir.ActivationFunctionType.Identity,
                            bias=negmr2[:], scale=rstd2[:],
                        )
                    final = pT.tile([128, N], F32, tag="pT")
                    nc.vector.tensor_add(
                        final[:], normed[:],
                        x_sb[:, oc, :].bitcast(F32),
                    )
                    nc.sync.dma_start(
                        out_ext[s, 128 * oc:128 * (oc + 1), :], final[:]
                    )

    nc.compile()
    return nc


def _get_nc():
    if "nc" not in _CACHE:
        _CACHE["nc"] = build_nc()
    return _CACHE["nc"]


def _prep_in_maps(x, g_w, theta_w, phi_w, W_w):
    # stacked projection lhsT: [c, ci] chunks -> [128, KCH, 3, CI]
    w3 = np.stack(
        [theta_w.T.reshape(KCH, 128, CI), phi_w.T.reshape(KCH, 128, CI),
         g_w.T.reshape(KCH, 128, CI)],
        axis=2,
    )  # [KCH, 128, 3, CI]
    w3 = np.ascontiguousarray(w3.transpose(1, 0, 2, 3), dtype=np.float32)
    ww = np.ascontiguousarray(W_w.T, dtype=np.float32)  # [CI, C]
    xr = np.ascontiguousarray(x.reshape(B, C, N), dtype=np.float32)
    in_maps = []
    for c in range(N_CORES):
        in_maps.append({
            "x": xr[B_LOC * c:B_LOC * (c + 1)],
            "w3": w3,
            "ww": ww,
        })
    return in_maps


def kernel(x, g_w, g_b, theta_w, theta_b, phi_w, phi_b, W_w, W_b, **_ignored):
    # biases are mathematically dropped by the InstanceNorms
    nc = _get_nc()
    in_maps = _prep_in_maps(x, g_w, theta_w, phi_w, W_w)
    res = run_bass_kernel_spmd(nc, in_maps, core_ids=list(range(N_CORES)))
    outs = [res.results[c]["out"].reshape(B_LOC, C, H, W) for c in range(N_CORES)]
    return np.concatenate(outs, axis=0).astype(np.float32)


def _install_ntff_hook():
    """Provide antenv.axon_hooks if the image lacks it (see trn_boot.py)."""
    import types
    try:
        from antenv.axon_hooks import get_axon_ntff_profile_hook  # noqa: F401
        return
    except ImportError:
        pass
    import contextlib
    import ctypes

    so_path = "/opt/axon/libaxon_pjrt.so"
    lib = ctypes.CDLL(so_path)
    if not hasattr(lib, "axon_start_nrt_profile"):
        hook = None
    else:
        lib.axon_start_nrt_profile.argtypes = [
            ctypes.POINTER(ctypes.c_int64), ctypes.c_size_t]
        lib.axon_start_nrt_profile.restype = ctypes.c_int64
        lib.axon_stop_nrt_profile.argtypes = [ctypes.c_char_p]
        lib.axon_stop_nrt_profile.restype = ctypes.c_int64

        @contextlib.contextmanager
        def hook(output_dir, device_ids):
            import jax
            jax.devices()
            if device_ids:
                ids = (ctypes.c_int64 * len(device_ids))(*device_ids)
                rc = lib.axon_start_nrt_profile(ids, len(device_ids))
            else:
                rc = lib.axon_start_nrt_profile(None, 0)
            if rc != 0:
                raise RuntimeError(f"axon_start_nrt_profile rc={rc}")
            try:
                yield
            finally:
                n = lib.axon_stop_nrt_profile(str(output_dir).encode())
                if n <= 0:
                    raise RuntimeError(f"axon_stop_nrt_profile rc={n}")

    mod = types.ModuleType("antenv.axon_hooks")
    mod.get_axon_ntff_profile_hook = lambda: hook
    mod.set_axon_ntff_profile_hook = lambda h: None
    sys.modules["antenv.axon_hooks"] = mod


def run_traced(x, g_w, g_b, theta_w, theta_b, phi_w, phi_b, W_w, W_b, **_ignored):
    """Like kernel() but with NTFF profiling; returns (out, BassKernelResults)."""
    _install_ntff_hook()
    nc = _get_nc()
    in_maps = _prep_in_maps(x, g_w, theta_w, phi_w, W_w)
    res = run_bass_kernel_spmd(
        nc, in_maps, core_ids=list(range(N_CORES)), trace=True
    )
    outs = [res.results[c]["out"].reshape(B_LOC, C, H, W) for c in range(N_CORES)]
    return np.concatenate(outs, axis=0).astype(np.float32), res
